# revision 6
# baseline (speedup 1.0000x reference)
"""DynamicConv Trainium2 kernel.

Math (B=1, L=2048, D=128, E=128, F=8, K1=K2=3, M=K2*D=384):
  f   = u @ proj                                   [L, F]
  kp[l,e,m] = sum_{k1,fc} f_pad[l+k1-1,fc] * W[e,k1,fc,m] + b[e,m]
  out[l,e]  = sum_{d,k2} u_pad[l+k2-1,d] * kp[l,e,d*K2+k2]

Swapping the summation order avoids materializing kp ([L,E,M] ~ 400MB):
  A_j[l,e]   = sum_{m'} patches[l,m'] * W'[m', j, e]     (j = k1*F+fc, 24 terms)
  bias_t[l,e]= sum_{m'} patches[l,m'] * b'[m', e]
  out[l,e]   = sum_j f_tap[l,j] * A_j[l,e] + bias_t[l,e]
with patches[l, (k2,d)] = u_pad[l+k2-1, d] — i.e. the patch matrix transposed
is just 3 shifted copies of u^T, so each l-tile needs only 3 matmuls of
[128,128] x [128,424] accumulated in PSUM.  The 424 PSUM columns are
  e*25 + j   (j<24):  A_j[l,e]
  e*25 + 24        :  bias_t[l,e]
  400 + k1*8 + fc  :  f_tap[l, k1*8+fc]  (proj columns embedded in the rhs of
                      matmul k2==k1 only; the other two accumulate zeros)
The combine is one broadcast tensor_tensor multiply (f over e, stride-0 AP)
plus one segmented reduce over 25 (bias slot multiplied by a constant 1.0).

E is sharded 8 ways (16 channels/core); u is replicated.
"""

import numpy as np

B, L, D = 1, 2048, 128
E, F = 128, 8
K1, K2 = 3, 3
M = K2 * D
NCORES = 8
EL = E // NCORES          # 16 output channels per core
NJ = K1 * F               # 24 (k1, fc) pairs
NJ1 = NJ + 1              # 25: + bias slot
NA = EL * NJ1             # 400 A/bias columns
NW = NA + NJ              # 424 total psum columns
LT = 128                  # l-tile size
NT = L // LT              # 16 tiles


def _build_program():
    import concourse.bass as bass
    import concourse.bacc as bacc
    import concourse.tile as tile
    from concourse import mybir

    f32 = mybir.dt.float32
    nc = bacc.Bacc("TRN2", target_bir_lowering=False, debug=False)

    # One packed input: cols [0, L+2) = u_padT, then K2 chunks of NW weight
    # columns.  A single DMA -> a single semaphore -> the first matmul carries
    # exactly one sync wait (the PE LDWEIGHTS slot only has one).
    NIN = (L + 2) + K2 * NW
    in_dram = nc.dram_tensor("packed", [D, NIN], f32, kind="ExternalInput")
    o_dram = nc.dram_tensor("out", [L, EL], f32, kind="ExternalOutput")

    with tile.TileContext(nc) as tc:
        import contextlib

        with contextlib.ExitStack() as ctx:
            const_pool = ctx.enter_context(tc.tile_pool(name="const", bufs=1))
            psum_pool = ctx.enter_context(
                tc.tile_pool(name="psum", bufs=6, space="PSUM")
            )
            asbp = ctx.enter_context(tc.tile_pool(name="asb", bufs=3))
            prodp = ctx.enter_context(tc.tile_pool(name="prod", bufs=3))
            outp = ctx.enter_context(tc.tile_pool(name="outt", bufs=3))

            in_sb = const_pool.tile([D, NIN], f32)
            nc.gpsimd.dma_start(out=in_sb[:], in_=in_dram[:])
            u_sb = in_sb[:, 0 : L + 2]
            w0 = L + 2

            for t in range(NT):
                ps = psum_pool.tile([LT, NW], f32)
                for k in range(K2):
                    nc.tensor.matmul(
                        ps[:],
                        u_sb[:, t * LT + k : t * LT + k + LT],
                        in_sb[:, w0 + k * NW : w0 + (k + 1) * NW],
                        start=(k == 0),
                        stop=(k == K2 - 1),
                    )

                # ACT is the only PSUM reader (keeps the next matmul on this
                # PSUM slot at a single sync wait).  asb[:, 424] = 1.0 is the
                # bias multiplier slot.
                asb = asbp.tile([LT, NW + 1], f32)
                nc.gpsimd.memset(asb[:, NW : NW + 1], 1.0)
                nc.scalar.copy(out=asb[:, 0:NW], in_=ps[:])

                # prod[l, e, j'] = A[l, e, j'] * f_tap[l, j'] (f broadcast on e)
                f_ap = asb[:, NA : NW + 1]
                f_bcast = bass.AP(
                    tensor=f_ap.tensor,
                    offset=f_ap.offset,
                    ap=[f_ap.ap[0], [0, EL], [1, NJ1]],
                )
                prod = prodp.tile([LT, EL, NJ1], f32)
                nc.vector.tensor_tensor(
                    out=prod[:],
                    in0=asb[:, 0:NA].rearrange("p (e j) -> p e j", j=NJ1),
                    in1=f_bcast,
                    op=mybir.AluOpType.mult,
                )

                o_t = outp.tile([LT, EL], f32)
                nc.vector.reduce_sum(
                    out=o_t[:], in_=prod[:], axis=mybir.AxisListType.X
                )
                nc.sync.dma_start(
                    out=o_dram[t * LT : (t + 1) * LT, :], in_=o_t[:]
                )

    nc.compile()
    return nc


def _prep_inputs(u, proj, conv_w, conv_b):
    """Host-side layout prep: pure reshuffling, no FLOPs on input data."""
    u_padt = np.zeros((D, L + 2), np.float32)
    u_padt[:, 1 : L + 1] = np.ascontiguousarray(u[0].T)

    in_maps = []
    for c in range(NCORES):
        e0 = c * EL
        w_aug = np.zeros((K2, D, NW), np.float32)
        # conv weights: m = d*K2 + k2 (in_channel-major, tap-minor)
        cw = conv_w[e0 : e0 + EL].reshape(EL, K1, F, D, K2)
        # -> [k2, d, e, j=(k1,fc)]
        wmain = cw.transpose(4, 3, 0, 1, 2).reshape(K2, D, EL, NJ)
        wa = w_aug[:, :, :NA].reshape(K2, D, EL, NJ1)
        wa[:, :, :, :NJ] = wmain
        # bias at j' = 24 (multiplied by the constant-1 f slot)
        cb = conv_b[e0 : e0 + EL, 0, :, 0].reshape(EL, D, K2)
        wa[:, :, :, NJ] = cb.transpose(2, 1, 0)
        # proj columns: only in the k2 == k1 matmul
        for k in range(K2):
            w_aug[k, :, NA + k * F : NA + (k + 1) * F] = proj
        packed = np.concatenate(
            [u_padt, w_aug.transpose(1, 0, 2).reshape(D, K2 * NW)], axis=1
        )
        in_maps.append({"packed": np.ascontiguousarray(packed)})
    return in_maps


_PROGRAM_CACHE = {}


def kernel(
    u,
    kernel_params_feat_proj,
    kernel_params_conv_weights,
    kernel_params_conv_bias,
):
    from concourse.bass_utils import run_bass_kernel_spmd

    u = np.asarray(u, np.float32)
    proj = np.asarray(kernel_params_feat_proj, np.float32)
    conv_w = np.asarray(kernel_params_conv_weights, np.float32)
    conv_b = np.asarray(kernel_params_conv_bias, np.float32)

    if "nc" not in _PROGRAM_CACHE:
        _PROGRAM_CACHE["nc"] = _build_program()
    nc = _PROGRAM_CACHE["nc"]

    in_maps = _prep_inputs(u, proj, conv_w, conv_b)
    res = run_bass_kernel_spmd(nc, in_maps, list(range(NCORES)))

    out = np.empty((B, L, E), np.float32)
    for c in range(NCORES):
        out[0, :, c * EL : (c + 1) * EL] = res.results[c]["out"]
    return out


# revision 9
# speedup vs baseline: 1.5459x; 1.5459x over previous
"""DynamicConv Trainium2 kernel.

Math (B=1, L=2048, D=128, E=128, F=8, K1=K2=3, M=K2*D=384):
  f   = u @ proj                                   [L, F]
  kp[l,e,m] = sum_{k1,fc} f_pad[l+k1-1,fc] * W[e,k1,fc,m] + b[e,m]
  out[l,e]  = sum_{d,k2} u_pad[l+k2-1,d] * kp[l,e,d*K2+k2]

Swapping the summation order avoids materializing kp ([L,E,M] ~ 400MB):
  A_j[l,e]   = sum_{m'} patches[l,m'] * W'[m', j, e]     (j = k1*F+fc, 24 terms)
  bias_t[l,e]= sum_{m'} patches[l,m'] * b'[m', e]
  out[l,e]   = sum_j f_tap[l,j] * A_j[l,e] + bias_t[l,e]
with patches[l, (k2,d)] = u_pad[l+k2-1, d] — i.e. the patch matrix transposed
is just 3 shifted copies of u^T, so each l-tile needs only 3 matmuls of
[128,128] x [128,424] accumulated in PSUM.  The 424 PSUM columns are
  e*25 + j   (j<24):  A_j[l,e]
  e*25 + 24        :  bias_t[l,e]
  400 + k1*8 + fc  :  f_tap[l, k1*8+fc]  (proj columns embedded in the rhs of
                      matmul k2==k1 only; the other two accumulate zeros)
The combine is one broadcast tensor_tensor multiply (f over e, stride-0 AP)
plus one segmented reduce over 25 (bias slot multiplied by a constant 1.0).

E is sharded 8 ways (16 channels/core); u is replicated.
"""

import numpy as np

B, L, D = 1, 2048, 128
E, F = 128, 8
K1, K2 = 3, 3
M = K2 * D
NCORES = 8
EL = E // NCORES          # 16 output channels per core
NJ = K1 * F               # 24 (k1, fc) pairs
NJ1 = NJ + 1              # 25: + bias slot
NA = EL * NJ1             # 400 A/bias columns
NW = NA + NJ              # 424 total psum columns
LT = 128                  # l-tile size
NT = L // LT              # 16 tiles


def _build_program():
    import concourse.bass as bass
    import concourse.bacc as bacc
    import concourse.tile as tile
    from concourse import mybir

    f32 = mybir.dt.float32
    nc = bacc.Bacc("TRN2", target_bir_lowering=False, debug=False)

    # One packed input: cols [0, L+2) = u_padT, then K2 chunks of NW weight
    # columns.  A single DMA -> a single semaphore -> the first matmul carries
    # exactly one sync wait (the PE LDWEIGHTS slot only has one).
    NIN = (L + 2) + K2 * NW
    f32r = mybir.dt.float32r
    in_dram = nc.dram_tensor("packed", [D, NIN], f32r, kind="ExternalInput")
    o_dram = nc.dram_tensor("out", [L, EL], f32, kind="ExternalOutput")

    with tile.TileContext(nc) as tc:
        import contextlib

        with contextlib.ExitStack() as ctx:
            const_pool = ctx.enter_context(tc.tile_pool(name="const", bufs=1))
            psum_pool = ctx.enter_context(
                tc.tile_pool(name="psum", bufs=6, space="PSUM")
            )
            asbp = ctx.enter_context(tc.tile_pool(name="asb", bufs=3))
            prodp = ctx.enter_context(tc.tile_pool(name="prod", bufs=3))
            outp = ctx.enter_context(tc.tile_pool(name="outt", bufs=3))

            in_sb = const_pool.tile([D, NIN], f32r)
            nc.gpsimd.dma_start(out=in_sb[:], in_=in_dram[:])
            u_sb = in_sb[:, 0 : L + 2]
            w0 = L + 2

            for t in range(NT):
                ps = psum_pool.tile([LT, NW], f32)
                for k in range(K2):
                    nc.tensor.matmul(
                        ps[:],
                        u_sb[:, t * LT + k : t * LT + k + LT],
                        in_sb[:, w0 + k * NW : w0 + (k + 1) * NW],
                        start=(k == 0),
                        stop=(k == K2 - 1),
                    )

                # ACT is the only PSUM reader (keeps the next matmul on this
                # PSUM slot at a single sync wait).  asb[:, 424] = 1.0 is the
                # bias multiplier slot.
                asb = asbp.tile([LT, NW + 1], f32)
                nc.gpsimd.memset(asb[:, NW : NW + 1], 1.0)
                nc.scalar.copy(out=asb[:, 0:NW], in_=ps[:])

                # prod[l, e, j'] = A[l, e, j'] * f_tap[l, j'] (f broadcast on e)
                f_ap = asb[:, NA : NW + 1]
                f_bcast = bass.AP(
                    tensor=f_ap.tensor,
                    offset=f_ap.offset,
                    ap=[f_ap.ap[0], [0, EL], [1, NJ1]],
                )
                prod = prodp.tile([LT, EL, NJ1], f32)
                nc.vector.tensor_tensor(
                    out=prod[:],
                    in0=asb[:, 0:NA].rearrange("p (e j) -> p e j", j=NJ1),
                    in1=f_bcast,
                    op=mybir.AluOpType.mult,
                )

                o_t = outp.tile([LT, EL], f32)
                nc.vector.reduce_sum(
                    out=o_t[:], in_=prod[:], axis=mybir.AxisListType.X
                )
                nc.sync.dma_start(
                    out=o_dram[t * LT : (t + 1) * LT, :], in_=o_t[:]
                )

    nc.compile()
    return nc


def _prep_inputs(u, proj, conv_w, conv_b):
    """Host-side layout prep: pure reshuffling, no FLOPs on input data."""
    u_padt = np.zeros((D, L + 2), np.float32)
    u_padt[:, 1 : L + 1] = np.ascontiguousarray(u[0].T)

    in_maps = []
    for c in range(NCORES):
        e0 = c * EL
        w_aug = np.zeros((K2, D, NW), np.float32)
        # conv weights: m = d*K2 + k2 (in_channel-major, tap-minor)
        cw = conv_w[e0 : e0 + EL].reshape(EL, K1, F, D, K2)
        # -> [k2, d, e, j=(k1,fc)]
        wmain = cw.transpose(4, 3, 0, 1, 2).reshape(K2, D, EL, NJ)
        wa = w_aug[:, :, :NA].reshape(K2, D, EL, NJ1)
        wa[:, :, :, :NJ] = wmain
        # bias at j' = 24 (multiplied by the constant-1 f slot)
        cb = conv_b[e0 : e0 + EL, 0, :, 0].reshape(EL, D, K2)
        wa[:, :, :, NJ] = cb.transpose(2, 1, 0)
        # proj columns: only in the k2 == k1 matmul
        for k in range(K2):
            w_aug[k, :, NA + k * F : NA + (k + 1) * F] = proj
        packed = np.concatenate(
            [u_padt, w_aug.transpose(1, 0, 2).reshape(D, K2 * NW)], axis=1
        )
        in_maps.append({"packed": np.ascontiguousarray(packed)})
    return in_maps


_PROGRAM_CACHE = {}


def kernel(
    u,
    kernel_params_feat_proj,
    kernel_params_conv_weights,
    kernel_params_conv_bias,
):
    from concourse.bass_utils import run_bass_kernel_spmd

    u = np.asarray(u, np.float32)
    proj = np.asarray(kernel_params_feat_proj, np.float32)
    conv_w = np.asarray(kernel_params_conv_weights, np.float32)
    conv_b = np.asarray(kernel_params_conv_bias, np.float32)

    if "nc" not in _PROGRAM_CACHE:
        _PROGRAM_CACHE["nc"] = _build_program()
    nc = _PROGRAM_CACHE["nc"]

    in_maps = _prep_inputs(u, proj, conv_w, conv_b)
    res = run_bass_kernel_spmd(nc, in_maps, list(range(NCORES)))

    out = np.empty((B, L, E), np.float32)
    for c in range(NCORES):
        out[0, :, c * EL : (c + 1) * EL] = res.results[c]["out"]
    return out


# revision 12
# speedup vs baseline: 1.7206x; 1.1130x over previous
"""DynamicConv Trainium2 kernel.

Math (B=1, L=2048, D=128, E=128, F=8, K1=K2=3, M=K2*D=384):
  f   = u @ proj                                   [L, F]
  kp[l,e,m] = sum_{k1,fc} f_pad[l+k1-1,fc] * W[e,k1,fc,m] + b[e,m]
  out[l,e]  = sum_{d,k2} u_pad[l+k2-1,d] * kp[l,e,d*K2+k2]

Swapping the summation order avoids materializing kp ([L,E,M] ~ 400MB):
  A_j[l,e]   = sum_{m'} patches[l,m'] * W'[m', j, e]     (j = k1*F+fc, 24 terms)
  bias_t[l,e]= sum_{m'} patches[l,m'] * b'[m', e]
  out[l,e]   = sum_j f_tap[l,j] * A_j[l,e] + bias_t[l,e]
with patches[l, (k2,d)] = u_pad[l+k2-1, d] — the patch matrix transposed is
just 3 shifted copies of u^T, so each l-tile of 128 positions needs only 3
bf16 matmuls of [128,128] x [128,424] accumulated in PSUM.  PSUM columns:
  e*25 + j   (j<24):  A_j[l,e]
  e*25 + 24        :  bias_t[l,e]
  400 + k1*8 + fc  :  f_tap[l, k1*8+fc]  (proj columns embedded in the rhs of
                      matmul k2==k1 only; the other two accumulate zeros)
Combine: ACT copies the 24 f columns to SBUF (+ a constant-1.0 col for the
bias slot), DVE does one broadcast multiply (f over e, stride-0 AP) and one
segmented reduce over 25.  Outputs are batched 8 l-tiles per DMA so each DMA
descriptor is 512B instead of 64B; the host un-permutes.

E is sharded 8 ways (16 channels/core); u is replicated.
"""

import numpy as np
import ml_dtypes

BF16 = ml_dtypes.bfloat16

B, L, D = 1, 2048, 128
E, F = 128, 8
K1, K2 = 3, 3
M = K2 * D
NCORES = 8
EL = E // NCORES          # 16 output channels per core
NJ = K1 * F               # 24 (k1, fc) pairs
NJ1 = NJ + 1              # 25: + bias slot
NA = EL * NJ1             # 400 A/bias columns
NW = NA + NJ              # 424 total psum columns
LT = 128                  # l-tile size
NT = L // LT              # 16 l-tiles
GT = 8                    # l-tiles per output DMA group
NG = NT // GT             # output groups
UC = 4                    # l-tiles per u chunk
UCOLS = UC * LT + 2       # 514
NU = NT // UC             # 4 u chunks


def _build_program():
    import concourse.bass as bass
    import concourse.bacc as bacc
    import concourse.tile as tile
    from concourse import mybir

    f32 = mybir.dt.float32
    bf16 = mybir.dt.bfloat16
    nc = bacc.Bacc("TRN2", target_bir_lowering=False, debug=False)

    u_dram = nc.dram_tensor("u_padt", [D, L + 2], bf16, kind="ExternalInput")
    w_dram = nc.dram_tensor("w_aug", [D, K2 * NW], bf16, kind="ExternalInput")
    o_dram = nc.dram_tensor("out", [NG, D, GT * EL], f32, kind="ExternalOutput")

    with tile.TileContext(nc) as tc:
        import contextlib

        with contextlib.ExitStack() as ctx:
            const_pool = ctx.enter_context(tc.tile_pool(name="const", bufs=1))
            psum_pool = ctx.enter_context(
                tc.tile_pool(name="psum", bufs=6, space="PSUM")
            )
            fpool = ctx.enter_context(tc.tile_pool(name="ftile", bufs=4))
            prodp = ctx.enter_context(tc.tile_pool(name="prod", bufs=4))
            outp = ctx.enter_context(tc.tile_pool(name="outt", bufs=2))

            w_sb = const_pool.tile([D, K2 * NW], bf16)
            nc.gpsimd.dma_start(out=w_sb[:], in_=w_dram[:])

            u_sbs = []
            for g in range(NU):
                u_g = const_pool.tile([D, UCOLS], bf16, tag=f"u{g}")
                nc.sync.dma_start(
                    out=u_g[:], in_=u_dram[:, g * UC * LT : g * UC * LT + UCOLS]
                )
                u_sbs.append(u_g)

            for gout in range(NG):
                o_big = outp.tile([LT, GT, EL], f32)
                for ti in range(GT):
                    t = gout * GT + ti
                    u_g = u_sbs[t // UC]
                    lo = (t % UC) * LT
                    ps = psum_pool.tile([LT, NW], f32)
                    for k in range(K2):
                        nc.tensor.matmul(
                            ps[:],
                            u_g[:, lo + k : lo + k + LT],
                            w_sb[:, k * NW : (k + 1) * NW],
                            start=(k == 0),
                            stop=(k == K2 - 1),
                        )

                    # f_tap [128, 25]: 24 taps + constant-1.0 bias slot
                    fsb = fpool.tile([LT, NJ1], f32)
                    nc.gpsimd.memset(fsb[:, NJ:NJ1], 1.0)
                    nc.scalar.copy(out=fsb[:, 0:NJ], in_=ps[:, NA:NW])

                    # prod[l, e, j'] = A[l, e, j'] * f_tap[l, j']
                    f_ap = fsb[:]
                    f_bcast = bass.AP(
                        tensor=f_ap.tensor,
                        offset=f_ap.offset,
                        ap=[f_ap.ap[0], [0, EL], [1, NJ1]],
                    )
                    prod = prodp.tile([LT, EL, NJ1], bf16)
                    nc.vector.tensor_tensor(
                        out=prod[:],
                        in0=ps[:, 0:NA].rearrange("p (e j) -> p e j", j=NJ1),
                        in1=f_bcast,
                        op=mybir.AluOpType.mult,
                    )
                    nc.vector.reduce_sum(
                        out=o_big[:, ti, :], in_=prod[:], axis=mybir.AxisListType.X
                    )

                nc.sync.dma_start(out=o_dram[gout], in_=o_big[:])

    nc.compile()
    return nc


def _prep_inputs(u, proj, conv_w, conv_b):
    """Host-side layout prep: reshuffle + bf16 rounding only."""
    u_padt = np.zeros((D, L + 2), BF16)
    u_padt[:, 1 : L + 1] = np.ascontiguousarray(u[0].T).astype(BF16)

    in_maps = []
    for c in range(NCORES):
        e0 = c * EL
        w_aug = np.zeros((K2, D, NW), np.float32)
        # conv weights: m = d*K2 + k2 (in_channel-major, tap-minor)
        cw = conv_w[e0 : e0 + EL].reshape(EL, K1, F, D, K2)
        wmain = cw.transpose(4, 3, 0, 1, 2).reshape(K2, D, EL, NJ)
        wa = w_aug[:, :, :NA].reshape(K2, D, EL, NJ1)
        wa[:, :, :, :NJ] = wmain
        # bias at j' = 24 (multiplied by the constant-1 f slot)
        cb = conv_b[e0 : e0 + EL, 0, :, 0].reshape(EL, D, K2)
        wa[:, :, :, NJ] = cb.transpose(2, 1, 0)
        # proj columns: only in the k2 == k1 matmul
        for k in range(K2):
            w_aug[k, :, NA + k * F : NA + (k + 1) * F] = proj
        w_flat = w_aug.transpose(1, 0, 2).reshape(D, K2 * NW).astype(BF16)
        in_maps.append(
            {"u_padt": u_padt, "w_aug": np.ascontiguousarray(w_flat)}
        )
    return in_maps


_PROGRAM_CACHE = {}


def kernel(
    u,
    kernel_params_feat_proj,
    kernel_params_conv_weights,
    kernel_params_conv_bias,
):
    from concourse.bass_utils import run_bass_kernel_spmd

    u = np.asarray(u, np.float32)
    proj = np.asarray(kernel_params_feat_proj, np.float32)
    conv_w = np.asarray(kernel_params_conv_weights, np.float32)
    conv_b = np.asarray(kernel_params_conv_bias, np.float32)

    if "nc" not in _PROGRAM_CACHE:
        _PROGRAM_CACHE["nc"] = _build_program()
    nc = _PROGRAM_CACHE["nc"]

    in_maps = _prep_inputs(u, proj, conv_w, conv_b)
    res = run_bass_kernel_spmd(nc, in_maps, list(range(NCORES)))

    out = np.empty((B, L, E), np.float32)
    for c in range(NCORES):
        # o_dram [NG, 128, GT, EL] with l = (g*GT + t)*128 + l_sub
        arr = res.results[c]["out"].reshape(NG, LT, GT, EL)
        arr = arr.transpose(0, 2, 1, 3).reshape(L, EL)
        out[0, :, c * EL : (c + 1) * EL] = arr
    return out
